# revision 9
# baseline (speedup 1.0000x reference)
"""GQA (no RoPE) Trainium2 kernel, 8 NeuronCores.

Sharding: 2 batches x 4 group-pair shards (2 KV groups + their 8 query heads
per core). All projections computed locally from pre-transposed bf16 inputs;
attention in transposed (key-major) layout so softmax denominators fall out of
the attn@v matmul via an appended ones-column on V; AllGather of normalized
attention outputs within each batch's 4-core group; o_proj column-sharded
(no all-reduce needed).

Self-contained: hardcodes shapes B=2, S=1024, D=2048, G=8, HG=4, HD=64.
"""

import os
import sys

sys.path.insert(0, "/opt/trn_rl_repo")

import numpy as np
import ml_dtypes

import concourse.bass as bass
import concourse.mybir as mybir
import concourse.tile as tile
from concourse import bacc
from concourse import bass_utils

BF16 = mybir.dt.bfloat16
F32 = mybir.dt.float32
AF = mybir.ActivationFunctionType

B, S, D = 2, 1024, 2048
G, HG, HD = 8, 4, 64            # groups, heads/group, head dim
P = 128                          # partitions
NCORES = 8
GPC = 2                          # groups per core
CQ = GPC * HG * HD               # q channels per core = 512
CK = GPC * HD                    # k/v channels per core = 128
CO = D // 4                      # output cols per core = 512
DC = D // P                      # contract chunks = 16
SC = S // P                      # seq chunks = 8
SEG = 512                        # psum bank width in f32


def _build_nc():
    nc = bacc.Bacc(
        "TRN2",
        target_bir_lowering=False,
        debug=False,
        enable_asserts=False,
        num_devices=NCORES,
    )

    # ---- I/O ----
    qt = nc.dram_tensor("qt", [D, S], BF16, kind="ExternalInput").ap()
    kt = nc.dram_tensor("kt", [D, S], BF16, kind="ExternalInput").ap()
    vt = nc.dram_tensor("vt", [D, S], BF16, kind="ExternalInput").ap()
    wqt = nc.dram_tensor("wqt", [D, CQ], BF16, kind="ExternalInput").ap()
    wkt = nc.dram_tensor("wkt", [D, CK], BF16, kind="ExternalInput").ap()
    wvt = nc.dram_tensor("wvt", [D, CK], BF16, kind="ExternalInput").ap()
    wot = nc.dram_tensor("wot", [D, CO], BF16, kind="ExternalInput").ap()
    bo = nc.dram_tensor("bo", [1, CO], BF16, kind="ExternalInput").ap()
    tri = nc.dram_tensor("tri", [P, P], BF16, kind="ExternalInput").ap()
    out = nc.dram_tensor("out", [S, CO], F32, kind="ExternalOutput").ap()

    with tile.TileContext(nc) as tc:
        with (
            tc.tile_pool(name="consts", bufs=1) as cp,
            tc.tile_pool(name="res", bufs=1) as rp,
            tc.tile_pool(name="psA", bufs=2, space="PSUM") as psA,
            tc.tile_pool(name="psB", bufs=2, space="PSUM") as psB,
            tc.tile_pool(name="dram", bufs=1, space="DRAM") as dp,
        ):
            tri_sb = cp.tile([P, P], BF16)
            nc.sync.dma_start(tri_sb[:], tri[:])
            bo_sb = cp.tile([1, CO], BF16)
            nc.sync.dma_start(bo_sb[:], bo[:])
            ones_sb = cp.tile([1, P], BF16)
            nc.vector.memset(ones_sb[:], 1.0)

            # resident projection outputs; head-major with partition base 0
            # so every scores matmul sees lhsT/rhs at the same base partition
            qt_sb = rp.tile([HD, GPC * HG, S], BF16)   # q^T per head
            kt_sb = rp.tile([HD, GPC, S], BF16)        # k^T per group
            vaug = rp.tile([P, SC, GPC, HD + 1], BF16)  # v natural + ones col
            attn_sb = rp.tile([P, CQ // P, S], BF16)   # normalized attn^T local

            nc.vector.memset(vaug[:, :, :, HD:HD + 1], 1.0)

            # ---- load transposed activations & weights, d-chunked ----
            with tc.tile_pool(name="xt", bufs=1) as xp:
                kx = [xp.tile([P, S], BF16, name=f"kx{d}") for d in range(DC)]
                wk = [xp.tile([P, CK], BF16, name=f"wk{d}") for d in range(DC)]
                vx = [xp.tile([P, S], BF16, name=f"vx{d}") for d in range(DC)]
                wv = [xp.tile([P, CK], BF16, name=f"wv{d}") for d in range(DC)]
                qx = [xp.tile([P, S], BF16, name=f"qx{d}") for d in range(DC)]
                wq = [xp.tile([P, CQ], BF16, name=f"wq{d}") for d in range(DC)]
                for d in range(DC):
                    r = slice(d * P, (d + 1) * P)
                    nc.sync.dma_start(kx[d][:], kt[r, :])
                    nc.sync.dma_start(wk[d][:], wkt[r, :])
                for d in range(DC):
                    r = slice(d * P, (d + 1) * P)
                    nc.sync.dma_start(vx[d][:], vt[r, :])
                    nc.sync.dma_start(wv[d][:], wvt[r, :])
                for d in range(DC):
                    r = slice(d * P, (d + 1) * P)
                    nc.sync.dma_start(qx[d][:], qt[r, :])
                    nc.sync.dma_start(wq[d][:], wqt[r, :])

                # ---- k projection: k^T[ck, s] ----
                ps = psA.tile([P, S], F32, tag="psA")
                for seg in range(2):
                    cs = slice(seg * SEG, (seg + 1) * SEG)
                    for d in range(DC):
                        nc.tensor.matmul(
                            ps[:, cs], wk[d][:], kx[d][:, cs],
                            start=(d == 0), stop=(d == DC - 1),
                        )
                nc.scalar.copy(kt_sb[:, 0, :], ps[0:HD, :])
                nc.scalar.copy(kt_sb[:, 1, :], ps[HD:P, :])

                # ---- v projection: v[s, cv] natural, into vaug ----
                for sc in range(SC):
                    ss = slice(sc * P, (sc + 1) * P)
                    pv = psA.tile([P, P], F32, tag="psA")
                    for d in range(DC):
                        nc.tensor.matmul(
                            pv[:], vx[d][:, ss], wv[d][:],
                            start=(d == 0), stop=(d == DC - 1),
                        )
                    for gl in range(GPC):
                        nc.scalar.copy(
                            vaug[:, sc, gl, 0:HD],
                            pv[:, gl * HD:(gl + 1) * HD],
                        )

                # ---- q projection: q^T[cq, s] ----
                for mq in range(CQ // P):
                    ms = slice(mq * P, (mq + 1) * P)
                    pq = psA.tile([P, S], F32, tag="psA")
                    for seg in range(2):
                        cs = slice(seg * SEG, (seg + 1) * SEG)
                        for d in range(DC):
                            nc.tensor.matmul(
                                pq[:, cs], wq[d][:, ms], qx[d][:, cs],
                                start=(d == 0), stop=(d == DC - 1),
                            )
                    nc.scalar.copy(qt_sb[:, 2 * mq, :], pq[0:HD, :])
                    nc.scalar.copy(qt_sb[:, 2 * mq + 1, :], pq[HD:P, :])

            # ---- w_o^T + bias loads (overlap with attention) ----
            wo = [rp.tile([P, CO], BF16, name=f"wo{d}") for d in range(DC)]
            for d in range(DC):
                nc.sync.dma_start(wo[d][:], wot[d * P:(d + 1) * P, :])

            # ---- attention, head by head ----
            with tc.tile_pool(name="probs", bufs=3) as pp:
                for gl in range(GPC):
                    for hh in range(HG):
                        hidx = gl * HG + hh
                        # where this head's channels live in attn_sb (channel-major)
                        qrow = gl * HG * HD + hh * HD     # 0..448 step 64
                        qpart = slice(qrow % P, qrow % P + HD)
                        qfree = qrow // P

                        oa = psB.tile([HD + 1, S], F32, tag="psB")
                        for m in range(SC):
                            nq0 = m * P
                            sc_ps = psA.tile([P, S], F32, tag="psA")
                            # segments within [nq0, S) split at psum bank SEG
                            segs = []
                            if nq0 < SEG:
                                segs.append((nq0, SEG))
                                segs.append((SEG, S))
                            else:
                                segs.append((nq0, S))
                            for (a, b2) in segs:
                                nc.tensor.matmul(
                                    sc_ps[:, a:b2],
                                    kt_sb[:, gl, m * P:(m + 1) * P],
                                    qt_sb[:, hidx, a:b2],
                                    start=True, stop=True,
                                )
                            pr = pp.tile([P, S], BF16, tag="probs")
                            nc.scalar.activation(
                                pr[:, nq0:S], sc_ps[:, nq0:S], AF.Exp,
                                scale=1.0 / np.sqrt(HD),
                            )
                            # causal mask on the diagonal block
                            nc.vector.tensor_mul(
                                pr[:, nq0:nq0 + P], pr[:, nq0:nq0 + P], tri_sb[:]
                            )
                            for (a, b2) in segs:
                                nc.tensor.matmul(
                                    oa[:, a:b2],
                                    vaug[:, m, gl, :],
                                    pr[:, a:b2],
                                    start=(m == 0),
                                    stop=(m == SC - 1) or (b2 == SEG and m == 3),
                                )

                        # normalize: out / denom, write into attn_sb
                        rec = pp.tile([1, S], F32, tag="rec")
                        nc.vector.reciprocal(rec[:], oa[HD:HD + 1, :])
                        rbc = pp.tile([HD, S], F32, tag="rbc")
                        nc.gpsimd.partition_broadcast(rbc[:], rec[:])
                        nc.vector.tensor_mul(
                            attn_sb[qpart, qfree, :], oa[0:HD, :], rbc[:]
                        )

            # ---- AllGather normalized attn within batch group ----
            agin = dp.tile([CQ, S], BF16)
            agout = dp.tile([D, S], BF16)
            nc.sync.dma_start(
                agin.rearrange("(q p) n -> p q n", p=P), attn_sb[:]
            )
            nc.gpsimd.collective_compute(
                "AllGather",
                mybir.AluOpType.bypass,
                replica_groups=[[0, 1, 2, 3], [4, 5, 6, 7]],
                ins=[agin.opt()],
                outs=[agout.opt()],
            )

            # ---- o_proj: out[s, o] = attn_full^T.T @ w_o^T + b_o ----
            with tc.tile_pool(name="af", bufs=1) as ap_pool, \
                 tc.tile_pool(name="osb", bufs=3) as op:
                af = [ap_pool.tile([P, S], BF16, name=f"af{c}") for c in range(DC)]
                for c in range(DC):
                    nc.sync.dma_start(af[c][:], agout[c * P:(c + 1) * P, :])
                for sc in range(SC):
                    ss = slice(sc * P, (sc + 1) * P)
                    po = psB.tile([P, CO], F32, tag="psB")
                    nc.tensor.matmul(
                        po[:], ones_sb[:], bo_sb[:], start=True, stop=False,
                    )
                    for c in range(DC):
                        nc.tensor.matmul(
                            po[:], af[c][:, ss], wo[c][:],
                            start=False, stop=(c == DC - 1),
                        )
                    ot = op.tile([P, CO], F32, tag="osb")
                    nc.scalar.copy(ot[:], po[:])
                    nc.sync.dma_start(out[ss, :], ot[:])

    nc.compile()
    return nc


_nc_cache = None


def build_in_maps(inputs):
    Q = np.asarray(inputs["Q"], np.float32)
    K = np.asarray(inputs["K"], np.float32)
    V = np.asarray(inputs["V"], np.float32)
    w_q = np.asarray(inputs["w_q"], np.float32)
    w_k = np.asarray(inputs["w_k"], np.float32)
    w_v = np.asarray(inputs["w_v"], np.float32)
    w_o = np.asarray(inputs["w_o"], np.float32)
    b_o = np.asarray(inputs["b_o"], np.float32)

    bf = ml_dtypes.bfloat16
    tri = np.triu(np.ones((P, P), np.float32)).astype(bf)  # key i <= query j

    in_maps = []
    for c in range(NCORES):
        b, j = divmod(c, 4)
        in_maps.append({
            "qt": np.ascontiguousarray(Q[b].T).astype(bf),
            "kt": np.ascontiguousarray(K[b].T).astype(bf),
            "vt": np.ascontiguousarray(V[b].T).astype(bf),
            "wqt": np.ascontiguousarray(w_q[j * CQ:(j + 1) * CQ, :].T).astype(bf),
            "wkt": np.ascontiguousarray(w_k[j * CK:(j + 1) * CK, :].T).astype(bf),
            "wvt": np.ascontiguousarray(w_v[j * CK:(j + 1) * CK, :].T).astype(bf),
            "wot": np.ascontiguousarray(w_o[j * CO:(j + 1) * CO, :].T).astype(bf),
            "bo": b_o[None, j * CO:(j + 1) * CO].astype(bf),
            "tri": tri,
        })
    return in_maps


def kernel(**inputs):
    global _nc_cache
    in_maps = build_in_maps(inputs)
    if _nc_cache is None:
        _nc_cache = _build_nc()
    nc = _nc_cache

    trace = bool(int(os.environ.get("BASS_KERNEL_TRACE", "0")))
    res = bass_utils.run_bass_kernel_spmd(
        nc, in_maps, core_ids=list(range(NCORES)), trace=trace,
    )
    kernel.last_results = res

    out = np.empty((B, S, D), np.float32)
    for c in range(NCORES):
        b, j = divmod(c, 4)
        out[b][:, j * CO:(j + 1) * CO] = res.results[c]["out"]
    return out


# revision 15
# speedup vs baseline: 1.1557x; 1.1557x over previous
"""GQA (no RoPE) Trainium2 kernel, 8 NeuronCores.

Sharding: 2 batches x 4 group-pair shards (2 KV groups + their 8 query heads
per core). All projections computed locally from pre-transposed bf16 inputs;
attention in transposed (key-major) layout so softmax denominators fall out of
the attn@v matmul via an appended ones-column on V; AllGather of normalized
attention outputs within each batch's 4-core group; o_proj column-sharded
(no all-reduce needed).

Self-contained: hardcodes shapes B=2, S=1024, D=2048, G=8, HG=4, HD=64.
"""

import os
import sys

sys.path.insert(0, "/opt/trn_rl_repo")

import numpy as np
import ml_dtypes

import concourse.bass as bass
import concourse.mybir as mybir
import concourse.tile as tile
from concourse import bacc
from concourse import bass_utils

BF16 = mybir.dt.bfloat16
F32 = mybir.dt.float32
AF = mybir.ActivationFunctionType

B, S, D = 2, 1024, 2048
G, HG, HD = 8, 4, 64            # groups, heads/group, head dim
P = 128                          # partitions
NCORES = 8
GPC = 2                          # groups per core
CQ = GPC * HG * HD               # q channels per core = 512
CK = GPC * HD                    # k/v channels per core = 128
CO = D // 4                      # output cols per core = 512
DC = D // P                      # contract chunks = 16
SC = S // P                      # seq chunks = 8
SEG = 512                        # psum bank width in f32
AG_CHUNKS = 1                    # 1 = single AllGather, 4 = per q-block


def _build_nc():
    nc = bacc.Bacc(
        "TRN2",
        target_bir_lowering=False,
        debug=False,
        enable_asserts=False,
        num_devices=NCORES,
    )

    # ---- I/O ----
    qt = nc.dram_tensor("qt", [D, S], BF16, kind="ExternalInput").ap()
    kt = nc.dram_tensor("kt", [D, S], BF16, kind="ExternalInput").ap()
    vt = nc.dram_tensor("vt", [D, S], BF16, kind="ExternalInput").ap()
    wqt = nc.dram_tensor("wqt", [D, CQ], BF16, kind="ExternalInput").ap()
    wkt = nc.dram_tensor("wkt", [D, CK], BF16, kind="ExternalInput").ap()
    wvt = nc.dram_tensor("wvt", [D, CK], BF16, kind="ExternalInput").ap()
    wot = nc.dram_tensor("wot", [D, CO], BF16, kind="ExternalInput").ap()
    bo = nc.dram_tensor("bo", [1, CO], BF16, kind="ExternalInput").ap()
    tri = nc.dram_tensor("tri", [P, P], BF16, kind="ExternalInput").ap()
    out = nc.dram_tensor("out", [S, CO], F32, kind="ExternalOutput").ap()

    with tile.TileContext(nc) as tc:
        with (
            tc.tile_pool(name="consts", bufs=1) as cp,
            tc.tile_pool(name="res", bufs=1) as rp,
            tc.tile_pool(name="psA", bufs=2, space="PSUM") as psA,
            tc.tile_pool(name="psB", bufs=2, space="PSUM") as psB,
            tc.tile_pool(name="dram", bufs=1, space="DRAM") as dp,
        ):
            tri_sb = cp.tile([P, P], BF16)
            nc.sync.dma_start(tri_sb[:], tri[:])
            bo_sb = cp.tile([1, CO], BF16)
            nc.sync.dma_start(bo_sb[:], bo[:])
            ones_sb = cp.tile([1, P], BF16)
            nc.vector.memset(ones_sb[:], 1.0)

            # resident projection outputs; head-major with partition base 0
            # so every scores matmul sees lhsT/rhs at the same base partition
            qt_sb = rp.tile([HD, GPC * HG, S], BF16)   # q^T per head
            kt_sb = rp.tile([HD, GPC, S], BF16)        # k^T per group
            vaug = rp.tile([P, SC, GPC, HD + 1], BF16)  # v natural + ones col
            attn_sb = rp.tile([P, CQ // P, S], BF16)   # normalized attn^T local

            nc.vector.memset(vaug[:, :, :, HD:HD + 1], 1.0)

            # ---- load transposed activations & weights, d-chunked ----
            with tc.tile_pool(name="xt", bufs=1) as xp:
                kx = [xp.tile([P, S], BF16, name=f"kx{d}") for d in range(DC)]
                wk = [xp.tile([P, CK], BF16, name=f"wk{d}") for d in range(DC)]
                vx = [xp.tile([P, S], BF16, name=f"vx{d}") for d in range(DC)]
                wv = [xp.tile([P, CK], BF16, name=f"wv{d}") for d in range(DC)]
                qx = [xp.tile([P, S], BF16, name=f"qx{d}") for d in range(DC)]
                wq = [xp.tile([P, CQ], BF16, name=f"wq{d}") for d in range(DC)]
                for d in range(DC):
                    r = slice(d * P, (d + 1) * P)
                    nc.sync.dma_start(kx[d][:], kt[r, :])
                    nc.sync.dma_start(wk[d][:], wkt[r, :])
                for d in range(DC):
                    r = slice(d * P, (d + 1) * P)
                    nc.sync.dma_start(vx[d][:], vt[r, :])
                    nc.sync.dma_start(wv[d][:], wvt[r, :])
                for d in range(DC):
                    r = slice(d * P, (d + 1) * P)
                    nc.sync.dma_start(qx[d][:], qt[r, :])
                    nc.sync.dma_start(wq[d][:], wqt[r, :])

                # ---- k projection: k^T[ck, s] ----
                ps = psA.tile([P, S], F32, tag="psA")
                for seg in range(2):
                    cs = slice(seg * SEG, (seg + 1) * SEG)
                    for d in range(DC):
                        nc.tensor.matmul(
                            ps[:, cs], wk[d][:], kx[d][:, cs],
                            start=(d == 0), stop=(d == DC - 1),
                        )
                nc.scalar.copy(kt_sb[:, 0, :], ps[0:HD, :])
                nc.scalar.copy(kt_sb[:, 1, :], ps[HD:P, :])

                # ---- v projection: v[s, cv] natural, into vaug ----
                for sc in range(SC):
                    ss = slice(sc * P, (sc + 1) * P)
                    pv = psA.tile([P, P], F32, tag="psA")
                    for d in range(DC):
                        nc.tensor.matmul(
                            pv[:], vx[d][:, ss], wv[d][:],
                            start=(d == 0), stop=(d == DC - 1),
                        )
                    for gl in range(GPC):
                        nc.scalar.copy(
                            vaug[:, sc, gl, 0:HD],
                            pv[:, gl * HD:(gl + 1) * HD],
                        )

                # ---- q projection: q^T[cq, s] ----
                for mq in range(CQ // P):
                    ms = slice(mq * P, (mq + 1) * P)
                    pq = psA.tile([P, S], F32, tag="psA")
                    for seg in range(2):
                        cs = slice(seg * SEG, (seg + 1) * SEG)
                        for d in range(DC):
                            nc.tensor.matmul(
                                pq[:, cs], wq[d][:, ms], qx[d][:, cs],
                                start=(d == 0), stop=(d == DC - 1),
                            )
                    nc.scalar.copy(qt_sb[:, 2 * mq, :], pq[0:HD, :])
                    nc.scalar.copy(qt_sb[:, 2 * mq + 1, :], pq[HD:P, :])

            # ---- w_o^T + bias loads (overlap with attention) ----
            wo = [rp.tile([P, CO], BF16, name=f"wo{d}") for d in range(DC)]
            for d in range(DC):
                nc.sync.dma_start(wo[d][:], wot[d * P:(d + 1) * P, :])

            # ---- attention: head pairs interleaved; AllGather per q-block ----
            nqb = CQ // P
            qb_per_ag = nqb // AG_CHUNKS
            agin = [dp.tile([qb_per_ag * P, S], BF16, name=f"agin{q}")
                    for q in range(AG_CHUNKS)]
            agout = [dp.tile([4 * qb_per_ag * P, S], BF16, name=f"agout{q}")
                     for q in range(AG_CHUNKS)]

            def scores_segs(m):
                nq0 = m * P
                if nq0 < SEG:
                    return [(nq0, SEG), (SEG, S)]
                return [(nq0, S)]

            with tc.tile_pool(name="probs", bufs=4) as pp:
                for pair in range(CQ // P):
                    heads = (2 * pair, 2 * pair + 1)
                    oas = {}
                    prs = {}
                    for m in range(SC):
                        for h in heads:
                            gl = h // HG
                            sc_ps = psA.tile([P, S], F32, tag="psA",
                                             name=f"sc{h}_{m}")
                            for (a, b2) in scores_segs(m):
                                nc.tensor.matmul(
                                    sc_ps[:, a:b2],
                                    kt_sb[:, gl, m * P:(m + 1) * P],
                                    qt_sb[:, h, a:b2],
                                    start=True, stop=True,
                                )
                            pr = pp.tile([P, S], BF16, tag="probs",
                                         name=f"pr{h}_{m}")
                            nc.scalar.activation(
                                pr[:, m * P:S], sc_ps[:, m * P:S], AF.Exp,
                                scale=1.0 / np.sqrt(HD),
                            )
                            nc.vector.tensor_mul(
                                pr[:, m * P:(m + 1) * P],
                                pr[:, m * P:(m + 1) * P], tri_sb[:]
                            )
                            prs[h] = pr
                        for h in heads:
                            gl = h // HG
                            if m == 0:
                                oas[h] = psB.tile([HD + 1, S], F32, tag="psB",
                                                  name=f"oa{h}")
                            for (a, b2) in scores_segs(m):
                                nc.tensor.matmul(
                                    oas[h][:, a:b2],
                                    vaug[:, m, gl, :],
                                    prs[h][:, a:b2],
                                    start=(m == 0),
                                    stop=(m == SC - 1) or (b2 == SEG and m == 3),
                                )
                    # normalize both heads of the pair, fire this q-block's AG
                    for h in heads:
                        qrow = h * HD
                        qpart = slice(qrow % P, qrow % P + HD)
                        # custom-DVE recip misreads PSUM at base partition 64
                        # on HW — stage the denominator row to SBUF first
                        den = pp.tile([1, S], F32, tag="den")
                        nc.scalar.copy(den[:], oas[h][HD:HD + 1, :])
                        rec = pp.tile([1, S], F32, tag="rec")
                        nc.vector.reciprocal_approx_fast(rec[:], den[:])
                        rbc = pp.tile([HD, S], F32, tag="rbc")
                        nc.gpsimd.partition_broadcast(rbc[:], rec[:])
                        nc.vector.tensor_mul(
                            attn_sb[qpart, pair, :], oas[h][0:HD, :], rbc[:]
                        )
                    ag_idx, ag_off = divmod(pair, qb_per_ag)
                    nc.sync.dma_start(
                        agin[ag_idx][ag_off * P:(ag_off + 1) * P, :],
                        attn_sb[:, pair, :],
                    )
                    if ag_off == qb_per_ag - 1:
                        nc.gpsimd.collective_compute(
                            "AllGather",
                            mybir.AluOpType.bypass,
                            replica_groups=[[0, 1, 2, 3], [4, 5, 6, 7]],
                            ins=[agin[ag_idx].opt()],
                            outs=[agout[ag_idx].opt()],
                        )

            # ---- o_proj: out[s, o] = attn_full^T.T @ w_o^T + b_o ----
            with tc.tile_pool(name="af", bufs=1) as ap_pool, \
                 tc.tile_pool(name="osb", bufs=3) as op:
                af = [ap_pool.tile([P, S], BF16, name=f"af{c}") for c in range(DC)]
                for c in range(DC):
                    r, q = divmod(c, nqb)
                    ag_idx, qo = divmod(q, qb_per_ag)
                    row = r * qb_per_ag * P + qo * P
                    nc.sync.dma_start(
                        af[c][:], agout[ag_idx][row:row + P, :]
                    )
                for sc in range(SC):
                    ss = slice(sc * P, (sc + 1) * P)
                    po = psB.tile([P, CO], F32, tag="psB")
                    nc.tensor.matmul(
                        po[:], ones_sb[:], bo_sb[:], start=True, stop=False,
                    )
                    for c in range(DC):
                        nc.tensor.matmul(
                            po[:], af[c][:, ss], wo[c][:],
                            start=False, stop=(c == DC - 1),
                        )
                    ot = op.tile([P, CO], F32, tag="osb")
                    nc.scalar.copy(ot[:], po[:])
                    nc.sync.dma_start(out[ss, :], ot[:])

    nc.compile()
    return nc


_nc_cache = None


def build_in_maps(inputs):
    Q = np.asarray(inputs["Q"], np.float32)
    K = np.asarray(inputs["K"], np.float32)
    V = np.asarray(inputs["V"], np.float32)
    w_q = np.asarray(inputs["w_q"], np.float32)
    w_k = np.asarray(inputs["w_k"], np.float32)
    w_v = np.asarray(inputs["w_v"], np.float32)
    w_o = np.asarray(inputs["w_o"], np.float32)
    b_o = np.asarray(inputs["b_o"], np.float32)

    bf = ml_dtypes.bfloat16
    tri = np.triu(np.ones((P, P), np.float32)).astype(bf)  # key i <= query j

    in_maps = []
    for c in range(NCORES):
        b, j = divmod(c, 4)
        in_maps.append({
            "qt": np.ascontiguousarray(Q[b].T).astype(bf),
            "kt": np.ascontiguousarray(K[b].T).astype(bf),
            "vt": np.ascontiguousarray(V[b].T).astype(bf),
            "wqt": np.ascontiguousarray(w_q[j * CQ:(j + 1) * CQ, :].T).astype(bf),
            "wkt": np.ascontiguousarray(w_k[j * CK:(j + 1) * CK, :].T).astype(bf),
            "wvt": np.ascontiguousarray(w_v[j * CK:(j + 1) * CK, :].T).astype(bf),
            "wot": np.ascontiguousarray(w_o[j * CO:(j + 1) * CO, :].T).astype(bf),
            "bo": b_o[None, j * CO:(j + 1) * CO].astype(bf),
            "tri": tri,
        })
    return in_maps


def kernel(**inputs):
    global _nc_cache
    in_maps = build_in_maps(inputs)
    if _nc_cache is None:
        _nc_cache = _build_nc()
    nc = _nc_cache

    trace = bool(int(os.environ.get("BASS_KERNEL_TRACE", "0")))
    res = bass_utils.run_bass_kernel_spmd(
        nc, in_maps, core_ids=list(range(NCORES)), trace=trace,
    )
    kernel.last_results = res

    out = np.empty((B, S, D), np.float32)
    for c in range(NCORES):
        b, j = divmod(c, 4)
        out[b][:, j * CO:(j + 1) * CO] = res.results[c]["out"]
    return out


# revision 16
# speedup vs baseline: 1.3365x; 1.1565x over previous
"""GQA (no RoPE) Trainium2 kernel, 8 NeuronCores.

Sharding: 2 batches x 4 group-pair shards (2 KV groups + their 8 query heads
per core). All projections computed locally from pre-transposed bf16 inputs;
attention in transposed (key-major) layout so softmax denominators fall out of
the attn@v matmul via an appended ones-column on V; AllGather of normalized
attention outputs within each batch's 4-core group; o_proj column-sharded
(no all-reduce needed).

Self-contained: hardcodes shapes B=2, S=1024, D=2048, G=8, HG=4, HD=64.
"""

import os
import sys

sys.path.insert(0, "/opt/trn_rl_repo")

import numpy as np
import ml_dtypes

import concourse.bass as bass
import concourse.mybir as mybir
import concourse.tile as tile
from concourse import bacc
from concourse import bass_utils

BF16 = mybir.dt.bfloat16
F32 = mybir.dt.float32
AF = mybir.ActivationFunctionType

B, S, D = 2, 1024, 2048
G, HG, HD = 8, 4, 64            # groups, heads/group, head dim
P = 128                          # partitions
NCORES = 8
GPC = 2                          # groups per core
CQ = GPC * HG * HD               # q channels per core = 512
CK = GPC * HD                    # k/v channels per core = 128
CO = D // 4                      # output cols per core = 512
DC = D // P                      # contract chunks = 16
SC = S // P                      # seq chunks = 8
SEG = 512                        # psum bank width in f32
AG_CHUNKS = 4                    # 1 = single AllGather, 4 = per q-block


def _build_nc():
    nc = bacc.Bacc(
        "TRN2",
        target_bir_lowering=False,
        debug=False,
        enable_asserts=False,
        num_devices=NCORES,
    )

    # ---- I/O ----
    qt = nc.dram_tensor("qt", [D, S], BF16, kind="ExternalInput").ap()
    kt = nc.dram_tensor("kt", [D, S], BF16, kind="ExternalInput").ap()
    vt = nc.dram_tensor("vt", [D, S], BF16, kind="ExternalInput").ap()
    wqt = nc.dram_tensor("wqt", [D, CQ], BF16, kind="ExternalInput").ap()
    wkt = nc.dram_tensor("wkt", [D, CK], BF16, kind="ExternalInput").ap()
    wvt = nc.dram_tensor("wvt", [D, CK], BF16, kind="ExternalInput").ap()
    wot = nc.dram_tensor("wot", [D, CO], BF16, kind="ExternalInput").ap()
    bo = nc.dram_tensor("bo", [1, CO], BF16, kind="ExternalInput").ap()
    tri = nc.dram_tensor("tri", [P, P], BF16, kind="ExternalInput").ap()
    out = nc.dram_tensor("out", [S, CO], F32, kind="ExternalOutput").ap()

    with tile.TileContext(nc) as tc:
        with (
            tc.tile_pool(name="consts", bufs=1) as cp,
            tc.tile_pool(name="res", bufs=1) as rp,
            tc.tile_pool(name="psA", bufs=2, space="PSUM") as psA,
            tc.tile_pool(name="psB", bufs=2, space="PSUM") as psB,
            tc.tile_pool(name="dram", bufs=1, space="DRAM") as dp,
        ):
            tri_sb = cp.tile([P, P], BF16)
            nc.sync.dma_start(tri_sb[:], tri[:])
            bo_sb = cp.tile([1, CO], BF16)
            nc.sync.dma_start(bo_sb[:], bo[:])
            ones_sb = cp.tile([1, P], BF16)
            nc.vector.memset(ones_sb[:], 1.0)

            # resident projection outputs; head-major with partition base 0
            # so every scores matmul sees lhsT/rhs at the same base partition
            qt_sb = rp.tile([HD, GPC * HG, S], BF16)   # q^T per head
            kt_sb = rp.tile([HD, GPC, S], BF16)        # k^T per group
            vaug = rp.tile([P, SC, GPC, HD + 1], BF16)  # v natural + ones col
            attn_sb = rp.tile([P, CQ // P, S], BF16)   # normalized attn^T local

            nc.vector.memset(vaug[:, :, :, HD:HD + 1], 1.0)

            # ---- load transposed activations & weights, d-chunked ----
            with tc.tile_pool(name="xt", bufs=1) as xp:
                kx = [xp.tile([P, S], BF16, name=f"kx{d}") for d in range(DC)]
                wk = [xp.tile([P, CK], BF16, name=f"wk{d}") for d in range(DC)]
                vx = [xp.tile([P, S], BF16, name=f"vx{d}") for d in range(DC)]
                wv = [xp.tile([P, CK], BF16, name=f"wv{d}") for d in range(DC)]
                qx = [xp.tile([P, S], BF16, name=f"qx{d}") for d in range(DC)]
                wq = [xp.tile([P, CQ], BF16, name=f"wq{d}") for d in range(DC)]
                for d in range(DC):
                    r = slice(d * P, (d + 1) * P)
                    nc.sync.dma_start(kx[d][:], kt[r, :])
                    nc.sync.dma_start(wk[d][:], wkt[r, :])
                for d in range(DC):
                    r = slice(d * P, (d + 1) * P)
                    nc.sync.dma_start(vx[d][:], vt[r, :])
                    nc.sync.dma_start(wv[d][:], wvt[r, :])
                for d in range(DC):
                    r = slice(d * P, (d + 1) * P)
                    nc.sync.dma_start(qx[d][:], qt[r, :])
                    nc.sync.dma_start(wq[d][:], wqt[r, :])

                # ---- k projection: k^T[ck, s] ----
                ps = psA.tile([P, S], F32, tag="psA")
                for seg in range(2):
                    cs = slice(seg * SEG, (seg + 1) * SEG)
                    for d in range(DC):
                        nc.tensor.matmul(
                            ps[:, cs], wk[d][:], kx[d][:, cs],
                            start=(d == 0), stop=(d == DC - 1),
                        )
                nc.scalar.copy(kt_sb[:, 0, :], ps[0:HD, :])
                nc.scalar.copy(kt_sb[:, 1, :], ps[HD:P, :])

                # ---- v projection: v[s, cv] natural, into vaug ----
                for sc in range(SC):
                    ss = slice(sc * P, (sc + 1) * P)
                    pv = psA.tile([P, P], F32, tag="psA")
                    for d in range(DC):
                        nc.tensor.matmul(
                            pv[:], vx[d][:, ss], wv[d][:],
                            start=(d == 0), stop=(d == DC - 1),
                        )
                    for gl in range(GPC):
                        nc.scalar.copy(
                            vaug[:, sc, gl, 0:HD],
                            pv[:, gl * HD:(gl + 1) * HD],
                        )

                # ---- q projection: q^T[cq, s] ----
                for mq in range(CQ // P):
                    ms = slice(mq * P, (mq + 1) * P)
                    pq = psA.tile([P, S], F32, tag="psA")
                    for seg in range(2):
                        cs = slice(seg * SEG, (seg + 1) * SEG)
                        for d in range(DC):
                            nc.tensor.matmul(
                                pq[:, cs], wq[d][:, ms], qx[d][:, cs],
                                start=(d == 0), stop=(d == DC - 1),
                            )
                    nc.scalar.copy(qt_sb[:, 2 * mq, :], pq[0:HD, :])
                    nc.scalar.copy(qt_sb[:, 2 * mq + 1, :], pq[HD:P, :])

            # ---- w_o^T + bias loads (overlap with attention) ----
            wo = [rp.tile([P, CO], BF16, name=f"wo{d}") for d in range(DC)]
            for d in range(DC):
                nc.sync.dma_start(wo[d][:], wot[d * P:(d + 1) * P, :])

            # ---- attention: head pairs interleaved; AllGather per q-block ----
            nqb = CQ // P
            qb_per_ag = nqb // AG_CHUNKS
            agin = [dp.tile([qb_per_ag * P, S], BF16, name=f"agin{q}")
                    for q in range(AG_CHUNKS)]
            agout = [dp.tile([4 * qb_per_ag * P, S], BF16, name=f"agout{q}")
                     for q in range(AG_CHUNKS)]

            def scores_segs(m):
                nq0 = m * P
                if nq0 < SEG:
                    return [(nq0, SEG), (SEG, S)]
                return [(nq0, S)]

            with tc.tile_pool(name="probs", bufs=4) as pp:
                for pair in range(CQ // P):
                    heads = (2 * pair, 2 * pair + 1)
                    oas = {}
                    prs = {}
                    for m in range(SC):
                        for h in heads:
                            gl = h // HG
                            sc_ps = psA.tile([P, S], F32, tag="psA",
                                             name=f"sc{h}_{m}")
                            for (a, b2) in scores_segs(m):
                                nc.tensor.matmul(
                                    sc_ps[:, a:b2],
                                    kt_sb[:, gl, m * P:(m + 1) * P],
                                    qt_sb[:, h, a:b2],
                                    start=True, stop=True,
                                )
                            pr = pp.tile([P, S], BF16, tag="probs",
                                         name=f"pr{h}_{m}")
                            nc.scalar.activation(
                                pr[:, m * P:S], sc_ps[:, m * P:S], AF.Exp,
                                scale=1.0 / np.sqrt(HD),
                            )
                            nc.vector.tensor_mul(
                                pr[:, m * P:(m + 1) * P],
                                pr[:, m * P:(m + 1) * P], tri_sb[:]
                            )
                            prs[h] = pr
                        for h in heads:
                            gl = h // HG
                            if m == 0:
                                oas[h] = psB.tile([HD + 1, S], F32, tag="psB",
                                                  name=f"oa{h}")
                            for (a, b2) in scores_segs(m):
                                nc.tensor.matmul(
                                    oas[h][:, a:b2],
                                    vaug[:, m, gl, :],
                                    prs[h][:, a:b2],
                                    start=(m == 0),
                                    stop=(m == SC - 1) or (b2 == SEG and m == 3),
                                )
                    # normalize both heads of the pair, fire this q-block's AG
                    for h in heads:
                        qrow = h * HD
                        qpart = slice(qrow % P, qrow % P + HD)
                        # custom-DVE recip misreads PSUM at base partition 64
                        # on HW — stage the denominator row to SBUF first
                        den = pp.tile([1, S], F32, tag="den")
                        nc.scalar.copy(den[:], oas[h][HD:HD + 1, :])
                        rec = pp.tile([1, S], F32, tag="rec")
                        nc.vector.reciprocal_approx_fast(rec[:], den[:])
                        rbc = pp.tile([HD, S], F32, tag="rbc")
                        nc.gpsimd.partition_broadcast(rbc[:], rec[:])
                        nc.vector.tensor_mul(
                            attn_sb[qpart, pair, :], oas[h][0:HD, :], rbc[:]
                        )
                    ag_idx, ag_off = divmod(pair, qb_per_ag)
                    nc.sync.dma_start(
                        agin[ag_idx][ag_off * P:(ag_off + 1) * P, :],
                        attn_sb[:, pair, :],
                    )
                    if ag_off == qb_per_ag - 1:
                        nc.gpsimd.collective_compute(
                            "AllGather",
                            mybir.AluOpType.bypass,
                            replica_groups=[[0, 1, 2, 3], [4, 5, 6, 7]],
                            ins=[agin[ag_idx].opt()],
                            outs=[agout[ag_idx].opt()],
                        )

            # ---- o_proj: out[s, o] = attn_full^T.T @ w_o^T + b_o ----
            with tc.tile_pool(name="af", bufs=1) as ap_pool, \
                 tc.tile_pool(name="osb", bufs=3) as op:
                af = [ap_pool.tile([P, S], BF16, name=f"af{c}") for c in range(DC)]
                for c in range(DC):
                    r, q = divmod(c, nqb)
                    ag_idx, qo = divmod(q, qb_per_ag)
                    row = r * qb_per_ag * P + qo * P
                    nc.sync.dma_start(
                        af[c][:], agout[ag_idx][row:row + P, :]
                    )
                for sc in range(SC):
                    ss = slice(sc * P, (sc + 1) * P)
                    po = psB.tile([P, CO], F32, tag="psB")
                    nc.tensor.matmul(
                        po[:], ones_sb[:], bo_sb[:], start=True, stop=False,
                    )
                    for c in range(DC):
                        nc.tensor.matmul(
                            po[:], af[c][:, ss], wo[c][:],
                            start=False, stop=(c == DC - 1),
                        )
                    ot = op.tile([P, CO], F32, tag="osb")
                    nc.scalar.copy(ot[:], po[:])
                    nc.sync.dma_start(out[ss, :], ot[:])

    nc.compile()
    return nc


_nc_cache = None


def build_in_maps(inputs):
    Q = np.asarray(inputs["Q"], np.float32)
    K = np.asarray(inputs["K"], np.float32)
    V = np.asarray(inputs["V"], np.float32)
    w_q = np.asarray(inputs["w_q"], np.float32)
    w_k = np.asarray(inputs["w_k"], np.float32)
    w_v = np.asarray(inputs["w_v"], np.float32)
    w_o = np.asarray(inputs["w_o"], np.float32)
    b_o = np.asarray(inputs["b_o"], np.float32)

    bf = ml_dtypes.bfloat16
    tri = np.triu(np.ones((P, P), np.float32)).astype(bf)  # key i <= query j

    in_maps = []
    for c in range(NCORES):
        b, j = divmod(c, 4)
        in_maps.append({
            "qt": np.ascontiguousarray(Q[b].T).astype(bf),
            "kt": np.ascontiguousarray(K[b].T).astype(bf),
            "vt": np.ascontiguousarray(V[b].T).astype(bf),
            "wqt": np.ascontiguousarray(w_q[j * CQ:(j + 1) * CQ, :].T).astype(bf),
            "wkt": np.ascontiguousarray(w_k[j * CK:(j + 1) * CK, :].T).astype(bf),
            "wvt": np.ascontiguousarray(w_v[j * CK:(j + 1) * CK, :].T).astype(bf),
            "wot": np.ascontiguousarray(w_o[j * CO:(j + 1) * CO, :].T).astype(bf),
            "bo": b_o[None, j * CO:(j + 1) * CO].astype(bf),
            "tri": tri,
        })
    return in_maps


def kernel(**inputs):
    global _nc_cache
    in_maps = build_in_maps(inputs)
    if _nc_cache is None:
        _nc_cache = _build_nc()
    nc = _nc_cache

    trace = bool(int(os.environ.get("BASS_KERNEL_TRACE", "0")))
    res = bass_utils.run_bass_kernel_spmd(
        nc, in_maps, core_ids=list(range(NCORES)), trace=trace,
    )
    kernel.last_results = res

    out = np.empty((B, S, D), np.float32)
    for c in range(NCORES):
        b, j = divmod(c, 4)
        out[b][:, j * CO:(j + 1) * CO] = res.results[c]["out"]
    return out
